# revision 23
# baseline (speedup 1.0000x reference)
"""Trainium2 Bass kernel for a pre-norm transformer block with banded
(sliding-window) attention.

Reference computation (B=4, T=2048, D=512, H=8 heads, head_dim=64,
FFN=2048, fp32):
    xn = rmsnorm(x) ; qkv = xn @ w_qkv ; banded attention (|q-k| <= 64)
    x  = x + attn_out @ w_out + b_out
    h  = gelu(rmsnorm(x) @ w1 + b1) ; out = x + h @ w2 + b2

Sharding: token-parallel over 8 NeuronCores.  B*T = 8192 tokens -> 1024
tokens per core (each core takes half of one batch row).  Because the
attention is banded with context <= 64, each shard only needs a
64-token halo on each side; row edges are zero-padded and masked.  No
collectives.

v2 design: FEATURE-MAJOR end to end.  x arrives pre-transposed from the
host ([feat, tok]); rmsnorm statistics are partition-dim reductions done
with an all-ones stationary matmul on the PE (which broadcasts the sum
across partitions for free); out_proj and FFN2 keep weights stationary
so their outputs stay feature-major; the residual stream lives
feature-major in SBUF (x is read from HBM exactly once); norm gains and
biases are per-partition scalars riding scalar_tensor_tensor / rank-1
matmul terms.  The final [feat, tok] output is un-transposed on the
host.  This removes every PE transpose and PSUM->SBUF copy of the v1
token-major design plus ~3MB of HBM traffic (broadcast constants, x
re-read).

Other key points (kept from v1):
  - qkv and both FFN matmuls run in fp8e4 with DoubleRow perf mode
    (weights x16-scaled into fp8 on the host, rescaled at eviction);
  - attention scores are computed TRANSPOSED (S.T[k, q]), k-block-major,
    with q stored twice (qfA/qfB, complementary head halves zeroed) so
    both heads of a pair stream against one shared kf stationary;
  - the band mask is multiplicative 0/1 bf16, applied on GpSimd after
    the exp; softmax row-sums come from an all-ones stationary matmul;
    normalization uses reciprocal_approx_fast (single custom-DVE op,
    ~5x faster than InstReciprocal) fused into the PSUM eviction.
"""

import numpy as np
import copy as _copy

import concourse.bass as bass
import concourse.tile as tile
from concourse import mybir
from concourse.bass_utils import run_bass_kernel_spmd

F32 = mybir.dt.float32
BF16 = mybir.dt.bfloat16
FP8 = mybir.dt.float8e4
DR = mybir.MatmulPerfMode.DoubleRow
AX = mybir.AxisListType.X
AF = mybir.ActivationFunctionType
MUL = mybir.AluOpType.mult
ADD = mybir.AluOpType.add
PSUM = bass.MemorySpace.PSUM

B, T, D = 4, 2048, 512
H, HD = 8, 64
FFN = 2048
EPS = 1e-6
TPC = 1024          # tokens per core
HALO = 64
TPAD = TPC + 2 * HALO   # 1152 padded tokens
NB = TPAD // 128        # 9 padded token blocks
NQ = TPC // 128         # 8 query blocks


def _split_waits(nc, maxw=1):
    """Stock walrus rejects instructions carrying more than `maxw` semaphore
    waits; move extras onto preceding no-ops on the same engine."""
    m = nc.m
    new_module = _copy.replace(m, functions=[])
    for function in m.functions:
        new_function = _copy.replace(function, blocks=[])
        new_function.set_allocations_from_list(function.allocations)
        for block in function.blocks:
            insts = []
            for inst in block.instructions:
                si = inst.sync_info
                if si is not None and len(si.on_wait) > maxw:
                    waits = list(si.on_wait)
                    extra, keep = waits[maxw:], waits[:maxw]
                    for j in range(0, len(extra), maxw):
                        insts.append(mybir.InstNoOp(
                            name=f"{inst.name}_wsplit{j}",
                            engine=inst.engine,
                            sync_info=mybir.SyncInfo(
                                on_wait=extra[j:j + maxw], on_update=[]),
                            bass_nofuse=True,
                        ))
                    inst.sync_info = mybir.SyncInfo(
                        on_wait=keep, on_update=list(si.on_update))
                insts.append(inst)
            new_function.blocks.append(_copy.replace(block, instructions=insts))
        new_module.functions.append(new_function)
    nc.m = new_module


def _build_nc():
    nc = bass.Bass("TRN2", debug=False)

    # x pre-transposed on host: xT[p, c, t] = x_padded[t, 128c + p]
    xt_d = nc.dram_tensor("xT", [128, 4, TPAD], BF16, kind="ExternalInput")
    masks_d = nc.dram_tensor("masks", [3, 128, 512], BF16, kind="ExternalInput")
    wqkv_d = nc.dram_tensor("w_qkv", [128, 2, 2, 3 * D], FP8,
                            kind="ExternalInput")
    wout_d = nc.dram_tensor("w_out", [128, 2, 2, D], FP8,
                            kind="ExternalInput")
    w1_d = nc.dram_tensor("w1", [128, 2, 2, FFN], FP8, kind="ExternalInput")
    w2_d = nc.dram_tensor("w2", [128, 8, 2, 4, 128], FP8, kind="ExternalInput")
    # per-partition scalar columns: b_out (4 chunks) + b1 (16)
    scal_d = nc.dram_tensor("scal", [128, 20], F32, kind="ExternalInput")
    # b2 * 16 and b_out * 16 as row vectors for rank-1 bias rides
    b2row_d = nc.dram_tensor("b2row", [1, 2, 4, 128], BF16,
                             kind="ExternalInput")
    out_d = nc.dram_tensor("out", [128, 4, TPC], F32, kind="ExternalOutput")

    with tile.TileContext(nc) as tc:
        with (
            tc.tile_pool(name="consts", bufs=1) as consts,
            tc.tile_pool(name="weights", bufs=1) as wpool,
            tc.tile_pool(name="acts", bufs=1) as acts,
            tc.tile_pool(name="scr", bufs=6) as scr,
            tc.tile_pool(name="sq", bufs=4) as sqp,
            tc.tile_pool(name="small", bufs=12) as small,
            tc.tile_pool(name="pt", bufs=3) as ptp,
            tc.tile_pool(name="psum_mm", bufs=3, space=PSUM) as psum_mm,
            tc.tile_pool(name="psum_sT", bufs=3, space=PSUM) as psum_sT,
            tc.tile_pool(name="psum_av", bufs=2, space=PSUM) as psum_av,
        ):
            # ---- constants + input loads -----------------------------------
            # x chunks go out first, one per engine DMA queue, so nothing is
            # queued ahead of them; weights follow on separate queues.
            xT = acts.tile([128, 4, TPAD], BF16, tag="xT")
            for s in range(3):
                sl = slice(384 * s, 384 * (s + 1))
                nc.sync.dma_start(xT[:, 0, sl], xt_d[:, 0, sl])
                nc.scalar.dma_start(xT[:, 2, sl], xt_d[:, 2, sl])
                nc.gpsimd.dma_start(xT[:, 1, sl], xt_d[:, 1, sl])
                nc.sync.dma_start(xT[:, 3, sl], xt_d[:, 3, sl])
            wqkv = wpool.tile([128, 2, 2, 3 * D], FP8, tag="wqkv")
            nc.sync.dma_start(wqkv[:, 0, :, :], wqkv_d[:, 0, :, :])
            nc.scalar.dma_start(wqkv[:, 1, :, :], wqkv_d[:, 1, :, :])
            scal = consts.tile([128, 20], F32, tag="scal")
            nc.gpsimd.dma_start(scal[:], scal_d[:])
            boutc = scal[:, 0:4]
            b1_fm = scal[:, 4:20]

            eps_t = consts.tile([128, 1], F32, tag="eps")
            nc.vector.memset(eps_t[:], EPS)
            zero_t = consts.tile([128, 1], F32, tag="zero")
            nc.vector.memset(zero_t[:], 0.0)
            ones64 = consts.tile([128, 64], BF16, tag="ones64")
            nc.vector.memset(ones64[:], 1.0)
            ones128 = consts.tile([128, 128], BF16, tag="ones128")
            nc.vector.memset(ones128[:], 1.0)
            ones_row = consts.tile([1, 512], BF16, tag="ones_row")
            nc.vector.memset(ones_row[:], 1.0)
            s16_t = consts.tile([128, 1], F32, tag="s16")
            nc.vector.memset(s16_t[:], 1.0 / 16)


            # PE warm-up: small matmuls keep the PE HAM activity window alive
            # while x lands and the rmsnorm chain runs, so the first real
            # matmuls hit the fast HAM clock.  Three of them chase the x
            # slice DMAs so the activity window spans the load phase.
            for s in range(3):
                psw = psum_av.tile([128, 2, 128], F32, tag="avrs",
                                   name=f"warmx{s}")
                nc.tensor.matmul(psw[0:64, 0, 0:64], ones64[:],
                                 xT[:, 0, 384 * s:384 * s + 64],
                                 start=True, stop=True)
            for wi in range(8):
                psw = psum_av.tile([128, 2, 128], F32, tag="avrs",
                                   name=f"warm{wi}")
                nc.tensor.matmul(psw[0:64, 0, 0:64], ones64[:], ones64[:],
                                 start=True, stop=True)

            # ---- phase 1: rmsnorm1, feature-major --------------------------
            # sum of squares over features = partition-dim reduction: square
            # per chunk (ACT/DVE split, bf16 out) then an all-ones stationary
            # matmul accumulating the 4 chunks; the PE broadcasts the sum
            # across partitions for free.  sqrt+eps on ACT, fast-approx
            # reciprocal on DVE, then normalize+gain+cast-to-fp8 in one
            # scalar_tensor_tensor per chunk (DVE/GpSimd split).
            rms1 = acts.tile([128, TPAD], F32, tag="rms1")
            inv1 = acts.tile([128, TPAD], F32, tag="inv1")
            xnT = acts.tile([128, 4, TPAD], FP8, tag="xnT")
            for j in range(3):
                sl = slice(384 * j, 384 * (j + 1))
                sq1 = []
                for c in range(4):
                    s = sqp.tile([128, 384], BF16, tag="sq",
                                 name=f"sq1_{c}")
                    if c < 2:
                        nc.scalar.activation(s[:], xT[:, c, sl], AF.Square,
                                             bias=zero_t[:])
                    else:
                        nc.vector.tensor_tensor(s[:], xT[:, c, sl],
                                                xT[:, c, sl], MUL)
                    sq1.append(s)
                ps = psum_sT.tile([128, 384], F32, tag="sT", name="rmsred")
                for c in range(4):
                    nc.tensor.matmul(ps[:], ones128[:], sq1[c][:],
                                     start=(c == 0), stop=(c == 3))
                nc.scalar.activation(rms1[:, sl], ps[:], AF.Sqrt,
                                     bias=eps_t[:], scale=1.0 / D)
                nc.vector.reciprocal_approx_fast(inv1[:, sl], rms1[:, sl])
                for c in range(4):
                    nc.vector.tensor_tensor(xnT[:, c, sl], xT[:, c, sl],
                                            inv1[:, sl], MUL)
                # spaced keep-warm matmul: rides the pipeline so the PE HAM
                # window never sees a fully idle 3.4us stretch
                psw = psum_av.tile([128, 2, 128], F32, tag="avrs",
                                   name=f"warmp{j}")
                nc.tensor.matmul(psw[0:64, 0, 0:64], ones64[:],
                                 xnT[:, 0, 384 * j:384 * j + 64],
                                 start=True, stop=True)

            # remaining weights: one batched DMA each, on queues that are
            # otherwise idle so they overlap the x loads
            wout = wpool.tile([128, 2, 2, D], FP8, tag="wout")
            nc.scalar.dma_start(wout[:], wout_d[:])
            w1b = wpool.tile([128, 2, 2, FFN], FP8, tag="w1")
            nc.scalar.dma_start(w1b[:], w1_d[:])
            w2b = wpool.tile([128, 8, 2, 4, 128], FP8, tag="w2")
            nc.sync.dma_start(w2b[:], w2_d[:])
            m_sb = consts.tile([128, 3, 512], BF16)
            nc.gpsimd.dma_start(m_sb[:], masks_d.rearrange("m p k -> p m k"))
            b2row = consts.tile([1, 2, 4, 128], BF16, tag="b2row")
            nc.gpsimd.dma_start(b2row[:], b2row_d[:])

            # ---- phase 2: qkv ---------------------------------------------
            # q, k feature-major [dim, tok]; v token-major [tok, dim].
            # q is stored twice with complementary head halves zeroed (qfA:
            # even head real / odd zero, qfB: the reverse) so each head's
            # banded-score matmul can stream a full-K=128 operand against the
            # SHARED kf stationary.  64 zero-padded columns on each side let
            # every query window be 256 wide.
            qfA = acts.tile([128, 4, TPAD + 128], BF16, tag="big")
            nc.gpsimd.memset(qfA[64:128, :, :], 0.0)
            nc.vector.memset(qfA[0:64, :, 0:64], 0.0)
            nc.vector.memset(qfA[0:64, :, TPAD + 64:TPAD + 128], 0.0)
            qfB = acts.tile([128, 4, TPAD + 128], BF16, tag="qfB")
            nc.gpsimd.memset(qfB[0:64, :, :], 0.0)
            nc.vector.memset(qfB[64:128, :, 0:64], 0.0)
            nc.vector.memset(qfB[64:128, :, TPAD + 64:TPAD + 128], 0.0)
            kf = acts.tile([128, 4, TPAD], BF16, tag="kf")

            def qkv_chunk(j):
                for m in range(8):
                    ps = psum_mm.tile([128, 384], F32, tag="mm")
                    for g in range(2):
                        nc.tensor.matmul(
                            ps[:], wqkv[:, g, :, 128 * m:128 * (m + 1)],
                            xnT[:, 2 * g:2 * g + 2, 384 * j:384 * (j + 1)],
                            start=(g == 0), stop=(g == 1), perf_mode=DR)
                    if m < 4:  # q: fold in 1/sqrt(head_dim) and the /16
                        nc.scalar.activation(
                            qfA[0:64, m, 64 + 384 * j:64 + 384 * (j + 1)],
                            ps[0:64, :], AF.Copy, scale=HD ** -0.5 / 16)
                        nc.vector.tensor_scalar_mul(
                            qfB[64:128, m, 64 + 384 * j:64 + 384 * (j + 1)],
                            ps[64:128, :], HD ** -0.5 / 16)
                    else:
                        nc.scalar.activation(
                            kf[:, m - 4, 384 * j:384 * (j + 1)], ps[:],
                            AF.Copy, scale=1.0 / 16)
            vt = acts.tile([128, NB, D], BF16, tag="vt")

            def v_block(i):
                ps = psum_mm.tile([128, 512], F32, tag="mm")
                for g in range(2):
                    nc.tensor.matmul(ps[:], xnT[:, 2 * g:2 * g + 2,
                                               128 * i:128 * (i + 1)],
                                     wqkv[:, g, :, 1024:1536],
                                     start=(g == 0), stop=(g == 1),
                                     perf_mode=DR)
                nc.vector.tensor_scalar_mul(vt[:, i, :], ps[:], 1.0 / 16)

            # ---- phase 3: banded attention, transposed scores --------------
            # Per key block j and head pair: S.T[k, q] over the 256-wide query
            # window.  exp on ScalarE straight from PSUM; multiplicative 0/1
            # band mask on GpSimd.  Per query block: AV (V stationary, P.T
            # streaming) and row-sums (ones stationary) on the PE; normalize
            # (approx reciprocal + multiply) fused with the PSUM->SBUF move.
            attn_f = acts.tile([128, 4, TPC], FP8, tag="a2")
            pt_tiles = {}

            def scores_block(j):
                msel = 0 if j == 0 else (2 if j == NB - 1 else 1)
                pt = ptp.tile([128, 4, 2, 256], BF16, tag="pt", name=f"pt{j}")
                pt_tiles[j] = pt
                for hp in range(4):
                    ps = psum_sT.tile([128, 2, 256], F32, tag="sT")
                    nc.tensor.matmul(
                        ps[:, 0, :], kf[:, hp, 128 * j:128 * (j + 1)],
                        qfA[:, hp, 128 * j:128 * j + 256],
                        start=True, stop=True)
                    nc.tensor.matmul(
                        ps[:, 1, :], kf[:, hp, 128 * j:128 * (j + 1)],
                        qfB[:, hp, 128 * j:128 * j + 256],
                        start=True, stop=True)
                    ptE = scr.tile([128, 2, 256], BF16, tag="ptE")
                    nc.scalar.activation(ptE[:], ps[:], AF.Exp, bias=zero_t[:])
                    eng = nc.gpsimd if (hp < 3 and 0 < j < NB - 1) else nc.vector
                    eng.tensor_tensor(pt[:, hp, :, :], ptE[:],
                                      m_sb[:, msel, :], MUL)

            def av_block(qb):
                for hp in range(4):
                    ps2 = psum_av.tile([128, 2, 128], F32, tag="avrs")
                    # complete the row-sum group before starting the AV group:
                    # the accumulation-group state is bank-granular
                    for w in range(2):
                        sl = slice(128, 256) if w == 0 else slice(0, 128)
                        pt = pt_tiles[qb + w]
                        for hi in range(2):
                            nc.tensor.matmul(
                                ps2[64 * hi:64 * hi + 64, 1, :], ones64[:],
                                pt[:, hp, hi, sl],
                                start=(w == 0), stop=(w == 1),
                                skip_group_check=True)
                    for w in range(2):
                        sl = slice(128, 256) if w == 0 else slice(0, 128)
                        pt = pt_tiles[qb + w]
                        for hi in range(2):
                            h = 2 * hp + hi
                            nc.tensor.matmul(
                                ps2[64 * hi:64 * hi + 64, 0, :],
                                vt[:, qb + w, 64 * h:64 * (h + 1)],
                                pt[:, hp, hi, sl],
                                start=(w == 0), stop=(w == 1),
                                skip_group_check=True)
                    rcp = scr.tile([128, 128], F32, tag="rcp")
                    nc.vector.reciprocal_approx_fast(rcp[:], ps2[:, 1, :])
                    nc.vector.tensor_tensor(
                        attn_f[:, hp, 128 * qb:128 * (qb + 1)],
                        ps2[:, 0, :], rcp[:], MUL)

            # ---- phase 4: out-proj + residual + rmsnorm2, feature-major ----
            # wout chunks stationary, attn_f streaming -> output transposed
            # [out-feat, tok].  Eviction fuses bias (per-partition scalar) and
            # residual (resident xT) in one scalar_tensor_tensor.
            x2T = acts.tile([128, 4, TPC], F32, tag="x2T")
            xn2T = acts.tile([128, 4, TPC], FP8, tag="xn2T")

            def outproj(oc, h):
                sl = slice(512 * h, 512 * (h + 1))
                ps = psum_mm.tile([128, 512], F32, tag="mm")
                nc.tensor.matmul(ps[:], b2row[:, 1, oc, :], ones_row[:],
                                 start=True, stop=False)
                for g in range(2):
                    nc.tensor.matmul(
                        ps[:], wout[:, g, :, 128 * oc:128 * (oc + 1)],
                        attn_f[:, 2 * g:2 * g + 2, sl],
                        start=False, stop=(g == 1), perf_mode=DR)
                nc.vector.scalar_tensor_tensor(
                    x2T[:, oc, sl], ps[:], s16_t[:],
                    xT[:, oc, HALO + 512 * h:HALO + 512 * (h + 1)],
                    op0=MUL, op1=ADD)

            def rmsnorm2(h):
                sl = slice(512 * h, 512 * (h + 1))
                sq2 = []
                for c in range(4):
                    s = sqp.tile([128, 512], BF16, tag="sq", name=f"sq2_{c}")
                    nc.vector.tensor_tensor(s[:], x2T[:, c, sl],
                                            x2T[:, c, sl], MUL)
                    sq2.append(s)
                ps = psum_sT.tile([128, 512], F32, tag="sT", name="rmsred2")
                for c in range(4):
                    nc.tensor.matmul(ps[:], ones128[:], sq2[c][:],
                                     start=(c == 0), stop=(c == 3))
                rms2 = scr.tile([128, 512], F32, tag="rms2")
                nc.scalar.activation(rms2[:], ps[:], AF.Sqrt,
                                     bias=eps_t[:], scale=1.0 / D)
                inv2 = scr.tile([128, 512], F32, tag="inv2")
                nc.vector.reciprocal_approx_fast(inv2[:], rms2[:])
                for c in range(4):
                    eng = nc.vector if c < 2 else nc.gpsimd
                    eng.tensor_tensor(xn2T[:, c, sl], x2T[:, c, sl],
                                      inv2[:], MUL)

            # ---- phase 5: FFN (fp8 DoubleRow, weights x16 on host) ---------
            # token-half-major so FFN2 on the first half overlaps FFN1's
            # second half.  FFN2 keeps w2 stationary -> output feature-major;
            # b2 rides the accumulation group as a rank-1 (b2*16) x ones term;
            # eviction fuses the 1/16 rescale and the residual add.
            hf = acts.tile([128, 16, TPC], FP8, tag="big")
            outT = acts.tile([128, 4, TPC], F32, tag="outT")

            def ffn1(m, h):
                sl = slice(512 * h, 512 * (h + 1))
                ps0 = psum_mm.tile([128, 512], F32, tag="mm")
                for g in range(2):
                    nc.tensor.matmul(
                        ps0[:], w1b[:, g, :, 128 * m:128 * (m + 1)],
                        xn2T[:, 2 * g:2 * g + 2, sl],
                        start=(g == 0), stop=(g == 1), perf_mode=DR)
                nc.scalar.activation(hf[:, m, sl], ps0[:], AF.Gelu,
                                     bias=b1_fm[:, m:m + 1], scale=1.0 / 16)

            def ffn2(oc, h):
                sl = slice(512 * h, 512 * (h + 1))
                ps = psum_mm.tile([128, 512], F32, tag="mm")
                nc.tensor.matmul(ps[:], b2row[:, 0, oc, :], ones_row[:],
                                 start=True, stop=False)
                for g in range(8):
                    nc.tensor.matmul(ps[:], w2b[:, g, :, oc, :],
                                     hf[:, 2 * g:2 * g + 2, sl],
                                     start=False, stop=(g == 7),
                                     perf_mode=DR)
                nc.vector.scalar_tensor_tensor(
                    outT[:, oc, sl], ps[:], s16_t[:], x2T[:, oc, sl],
                    op0=MUL, op1=ADD)

            for j in range(3):
                qkv_chunk(j)
            for j in range(NB):
                v_block(j)
                scores_block(j)
                if j >= 2:
                    av_block(j - 2)
                if j == 6:
                    for oc in range(4):
                        outproj(oc, 0)
            rmsnorm2(0)
            av_block(NQ - 1)
            for oc in range(4):
                outproj(oc, 1)
            rmsnorm2(1)
            for m in range(16):
                ffn1(m, 0)
            for k in range(4):
                for m in range(4 * k, 4 * k + 4):
                    ffn1(m, 1)
                ffn2(k, 0)
                eng = (nc.sync, nc.scalar, nc.gpsimd, nc.sync)[k]
                eng.dma_start(out_d[:, k, 0:512], outT[:, k, 0:512])
            for oc in range(4):
                ffn2(oc, 1)
                eng = (nc.sync, nc.scalar, nc.gpsimd, nc.sync)[oc]
                eng.dma_start(out_d[:, oc, 512:1024], outT[:, oc, 512:1024])

    nc.finalize()
    # populate .instr bytes for the custom-DVE (reciprocal_approx_fast)
    # extended instructions; raw Bass skips this Bacc pass and the NEFF
    # compiler fails with "ISA wrong length" without it
    from concourse.library_overlay import lower_extended_insts
    lower_extended_insts(nc)
    if _DO_SPLIT_WAITS:
        _split_waits(nc)
    return nc


_DO_SPLIT_WAITS = True
_NC = None


def _get_nc():
    global _NC
    if _NC is None:
        _NC = _build_nc()
    return _NC


def _make_in_maps(x, norm1_w, norm2_w, w_qkv, w_out, b_out, w1, b1, w2, b2,
                  context_size):
    import ml_dtypes
    bf16 = ml_dtypes.bfloat16
    c = int(np.asarray(context_size))
    assert c <= HALO, f"context_size {c} exceeds compiled halo {HALO}"
    x = np.ascontiguousarray(np.asarray(x, np.float32))
    fp8 = ml_dtypes.float8_e4m3
    # qkv/ffn weights: x16 into fp8 range, DoubleRow layout
    # [partition, k-group, 2-interleave, out] with k = 128*(2g+e)+p
    n1 = np.asarray(norm1_w, np.float32)[:, None]
    n2 = np.asarray(norm2_w, np.float32)[:, None]
    w1_f8 = (np.asarray(w1, np.float32) * n2 * 16).reshape(2, 2, 128, FFN) \
        .transpose(2, 0, 1, 3).astype(fp8)
    # w2 FM: [p, g, e, oc, of] with k = 256g + 128e + p
    w2_f8 = (np.asarray(w2, np.float32) * 16).reshape(8, 2, 128, 4, 128) \
        .transpose(2, 0, 1, 3, 4).astype(fp8)
    wqkv_f8 = (np.asarray(w_qkv, np.float32) * n1 * 16) \
        .reshape(2, 2, 128, 3 * D).transpose(2, 0, 1, 3).astype(fp8)
    wout_fm = (np.asarray(w_out, np.float32) * 16).reshape(2, 2, 128, D) \
        .transpose(2, 0, 1, 3).astype(fp8)
    scal = np.empty((128, 20), np.float32)
    scal[:, 0:4] = np.asarray(b_out, np.float32).reshape(4, 128).T
    scal[:, 4:20] = np.asarray(b1, np.float32).reshape(16, 128).T
    b2row = np.stack([
        np.asarray(b2, np.float32).reshape(4, 128) * 16,
        np.asarray(b_out, np.float32).reshape(4, 128) * 16,
    ])[None].astype(bf16)
    shared = {
        "w_qkv": np.ascontiguousarray(wqkv_f8),
        "w_out": np.ascontiguousarray(wout_fm),
        "w1": np.ascontiguousarray(w1_f8),
        "w2": np.ascontiguousarray(w2_f8),
        "scal": np.ascontiguousarray(scal),
        "b2row": np.ascontiguousarray(b2row),
    }
    in_maps = []
    o = np.arange(128)[:, None]   # key offset within block (partition)
    u = np.arange(256)[None, :]   # query offset within 256-wide window
    for core in range(8):
        b, t0 = core // 2, (core % 2) * TPC
        lo, hi = t0 - HALO, t0 + TPC + HALO
        xp = np.zeros((TPAD, D), np.float32)
        s0, s1 = max(lo, 0), min(hi, T)
        xp[s0 - lo:s0 - lo + (s1 - s0)] = x[b, s0:s1]
        # feature-major: xT[p, c, t] = xp[t, 128c + p]
        xT = np.ascontiguousarray(
            xp.T.reshape(4, 128, TPAD).transpose(1, 0, 2)).astype(bf16)
        # Transposed multiplicative masks, k-block-major: maskT[o, u] guards
        # key 128j + o (partition) against query 128j - 64 + u (padded
        # coords); duplicated along the free dim for the two heads of a pair.
        masks = np.empty((3, 128, 256), np.float32)
        for mi, j in ((0, 0), (1, 3), (2, NB - 1)):
            kg = t0 - HALO + 128 * j + o
            qg = t0 - HALO + 128 * j - 64 + u
            ok = (np.abs(qg - kg) <= c) & (kg >= 0) & (kg < T) \
                & (qg >= 0) & (qg < T)
            masks[mi] = ok.astype(np.float32)
        masks = np.concatenate([masks, masks], axis=2)  # dup for head pairs
        in_maps.append({"xT": xT, "masks": masks.astype(bf16), **shared})
    return in_maps


def _run(in_maps, **kwargs):
    return run_bass_kernel_spmd(_get_nc(), in_maps, core_ids=list(range(8)),
                                **kwargs)


def kernel(**inputs):
    in_maps = _make_in_maps(**inputs)
    res = _run(in_maps)
    out = np.empty((B, T, D), np.float32)
    for core in range(8):
        b, t0 = core // 2, (core % 2) * TPC
        # arr[p, c, t] = out[t, 128c + p]
        arr = np.asarray(res.results[core]["out"])
        out[b, t0:t0 + TPC] = arr.transpose(2, 1, 0).reshape(TPC, D)
    return out
